# revision 22
# baseline (speedup 1.0000x reference)
"""Trainium2 Bass kernel for BiDenseAdjacency: y[b] = feats[b] @ w @ feats[b]^T + bias.

Full inputs in, full outputs out. Data-parallel over batch: B=32 batches split
4-per-core across 8 NeuronCores; w/b replicated. Per core and batch:
  T   = feats[b]^T                (PE transposes, [F=128, N=1024] in SBUF)
  vwT = w^T-contract:  vwT[g,n] = sum_f w[f,g] T[f,n]      (matmul, lhsT=w)
  y   = vwT^T-contract: y[n,m] = sum_g vwT[g,n] T[g,m] + b (matmul, lhsT=vwT)
Output DMA (16 MiB/core) is the roofline. Matmuls run as float32r (single-pass
fp32, ~2e-4 rel err) to keep the tensor engine off the critical path. Stores
go out in 2 MiB half-batches alternating between the SP and ACT HWDGE rings;
feats prefetches ride the GpSimd SWDGE path so they never block stores.
"""

import numpy as np

import concourse.mybir as mybir
import concourse.tile as tile
from concourse import bacc
from concourse.tile import add_dep_helper
from concourse.bass_utils import run_bass_kernel_spmd
from concourse.masks import make_identity

B, N, F = 32, 1024, 128
N_CORES = 8
BPC = B // N_CORES  # batches per core
P = 128             # partitions
NT = N // P         # row tiles per batch
H = 512             # matmul moving-dim chunk (PSUM bank limit for 4-byte)

F32 = mybir.dt.float32
MM_DT = mybir.dt.float32r  # matmul compute dtype (1 cyc/row vs 4 for float32)


def build_nc(mm_dt=MM_DT):
    nc = bacc.Bacc("TRN2", target_bir_lowering=False, debug=False,
                   num_devices=N_CORES)
    feats_d = nc.dram_tensor("feats", [BPC, N, F], F32, kind="ExternalInput")
    w_d = nc.dram_tensor("w", [F, F], F32, kind="ExternalInput")
    b_d = nc.dram_tensor("b", [1], F32, kind="ExternalInput")
    y_d = nc.dram_tensor("y", [BPC, N, N], F32, kind="ExternalOutput")

    with tile.TileContext(nc) as tc:
        with (
            tc.tile_pool(name="const", bufs=1) as cpool,
            tc.tile_pool(name="fload", bufs=BPC) as fpool,
            tc.tile_pool(name="tbuf", bufs=2) as tpool,
            tc.tile_pool(name="vwbuf", bufs=2) as vpool,
            tc.tile_pool(name="ybuf", bufs=4) as ypool,
            tc.tile_pool(name="ps_t", bufs=2, space="PSUM") as ps_t,
            tc.tile_pool(name="ps_y", bufs=2, space="PSUM") as ps_y,
        ):
            # Tiny loads first on each ring so mm1's weights aren't stuck
            # behind megabyte feats transfers in the ring FIFO.
            ident = cpool.tile([P, P], F32)
            make_identity(nc, ident[:])
            w_s = cpool.tile([F, F], F32)
            nc.gpsimd.dma_start(w_s[:], w_d[:])
            bias_s = cpool.tile([P, 1], F32)
            nc.gpsimd.dma_start(bias_s[:1, :], b_d[None, :])
            nc.gpsimd.partition_broadcast(bias_s[:], bias_s[:1, :])
            # matmul operands must be written in the matmul dtype (fp32r
            # rounding happens at the producing instruction per the BIR
            # verifier), so keep fp32r copies of w / T / vwT in SBUF.
            w_r = cpool.tile([F, F], mm_dt, tag="w_r")
            nc.vector.tensor_copy(w_r[:], w_s[:])


            # feats, contiguous layout: f_t[p, j, f] = feats[bi, p*NT + j, f]
            # (4 KiB contiguous per partition -> full-rate DMA). Batch 0 is
            # split in halves across both HWDGE rings to cut time-to-first-
            # transpose; later batches prefetch via SWDGE off the store path.
            f_tiles = []
            prev_loads = []
            for bi in range(BPC):
                f_t = fpool.tile([P, NT, F], F32, tag="f_t")
                fsrc = feats_d[bi].rearrange("(p j) f -> p j f", j=NT)
                if bi == 0:
                    prev_loads = []
                    for qi in range(4):
                        eng = nc.sync if qi % 2 == 0 else nc.scalar
                        dq = eng.dma_start(
                            f_t[:, qi * 2:(qi + 1) * 2, :],
                            fsrc[:, qi * 2:(qi + 1) * 2, :],
                        )
                        if qi < 2:
                            prev_loads.append(dq.ins)
                else:
                    # SWDGE path: own queues, so prefetches never head-of-line
                    # block the HWDGE store rings. Chained behind the prior
                    # load: the SDMA engines round-robin between queues at
                    # packet granularity, so concurrent loads would stretch
                    # batch 0's completion.
                    d = nc.gpsimd.dma_start(f_t[:], fsrc)
                    for pd in prev_loads:
                        add_dep_helper(d.ins, pd, reason="sequence feats prefetch")
                    prev_loads = [d.ins]
                f_tiles.append(f_t)

            def emit_transposes(bi):
                """Transpose feats[bi]^T into PSUM, scatter-copy to SBUF."""
                f_t = f_tiles[bi]
                t_ps = ps_t.tile([P, NT, P], F32, tag="t_ps")
                for j in range(NT):
                    nc.tensor.transpose(t_ps[:, j, :], f_t[:, j, :], ident[:])
                t_s = tpool.tile([P, N], mm_dt, tag="t_s")
                t_view = t_s[:].rearrange("g (p j) -> g j p", j=NT)
                nc.vector.tensor_copy(t_view[:, : NT // 2, :], t_ps[:, : NT // 2, :])
                nc.vector.tensor_copy(t_view[:, NT // 2:, :], t_ps[:, NT // 2:, :])
                return t_s

            t_cur = emit_transposes(0)
            for bi in range(BPC):
                t_s = t_cur
                # Transposes for the next batch go to the PE *before* this
                # batch's matmuls so the PE never idles on the T copies.
                if bi + 1 < BPC:
                    t_cur = emit_transposes(bi + 1)

                # vwT[g, n] = sum_f w[f, g] * T[f, n]
                vw_ps = ps_t.tile([P, N], F32, tag="t_ps")
                for h in range(N // H):
                    nc.tensor.matmul(
                        vw_ps[:, h * H:(h + 1) * H],
                        w_r[:],
                        t_s[:, h * H:(h + 1) * H],
                    )
                vw_s = vpool.tile([P, N], mm_dt)
                nc.scalar.activation(
                    vw_s[:, :H], vw_ps[:, :H], mybir.ActivationFunctionType.Copy
                )
                nc.vector.tensor_copy(vw_s[:, H:], vw_ps[:, H:])

                # y rows i*P..(i+1)*P-1 = vw_s[:, i*P:(i+1)*P].T @ T (+ bias)
                half_tiles = NT // 2
                for half in range(2):
                    y_s = ypool.tile([P, half_tiles, N], F32, tag="y_s")
                    for k in range(half_tiles):
                        i = half * half_tiles + k
                        y_ps = ps_y.tile([P, N], F32)
                        for h in range(N // H):
                            nc.tensor.matmul(
                                y_ps[:, h * H:(h + 1) * H],
                                vw_s[:, i * P:(i + 1) * P],
                                t_s[:, h * H:(h + 1) * H],
                            )
                        if i % 2 == 0:
                            nc.scalar.activation(
                                y_s[:, k, :], y_ps[:],
                                mybir.ActivationFunctionType.Identity,
                                bias=bias_s[:],
                            )
                        else:
                            nc.vector.tensor_scalar_add(
                                y_s[:, k, :], y_ps[:], bias_s[:]
                            )

                    dram_half = y_d[bi].rearrange("(i p) m -> p i m", p=P)[
                        :, half * half_tiles:(half + 1) * half_tiles, :
                    ]
                    if bi == 0 and half == 0:
                        # Per-tile chunks: the store stream starts immediately.
                        for qi in range(half_tiles):
                            eng = nc.sync if qi % 2 == 0 else nc.scalar
                            eng.dma_start(
                                dram_half[:, qi:qi + 1, :],
                                y_s[:, qi:qi + 1, :],
                            )
                    elif bi == BPC - 1:
                        # Last batch: quarter-chunks so the tail drains fast.
                        q = half_tiles // 2
                        for qi in range(2):
                            eng = nc.sync if (half + qi) % 2 == 0 else nc.scalar
                            eng.dma_start(
                                dram_half[:, qi * q:(qi + 1) * q, :],
                                y_s[:, qi * q:(qi + 1) * q, :],
                            )
                    else:
                        eng = nc.sync if half == 0 else nc.scalar
                        eng.dma_start(dram_half, y_s[:])

    nc.compile()
    return nc


_NC_CACHE = {}


def _get_nc(mm_dt=MM_DT):
    key = str(mm_dt)
    if key not in _NC_CACHE:
        _NC_CACHE[key] = build_nc(mm_dt)
    return _NC_CACHE[key]


def run_on_cores(feats, w, b, mm_dt=MM_DT, trace=False):
    nc = _get_nc(mm_dt)
    feats = np.ascontiguousarray(feats, dtype=np.float32)
    w = np.ascontiguousarray(w, dtype=np.float32)
    b = np.ascontiguousarray(b, dtype=np.float32)
    in_maps = [
        {"feats": feats[c * BPC:(c + 1) * BPC], "w": w, "b": b}
        for c in range(N_CORES)
    ]
    res = run_bass_kernel_spmd(nc, in_maps, core_ids=list(range(N_CORES)),
                               trace=trace)
    y = np.concatenate([res.results[c]["y"] for c in range(N_CORES)], axis=0)
    return y, res


def kernel(adjMs, feats, w, b):
    y, _ = run_on_cores(feats, w, b)
    feats = np.asarray(feats, dtype=np.float32)
    return y, feats


# revision 23
# speedup vs baseline: 1.1054x; 1.1054x over previous
"""Trainium2 Bass kernel for BiDenseAdjacency: y[b] = feats[b] @ w @ feats[b]^T + bias.

Full inputs in, full outputs out. Data-parallel over batch: B=32 batches split
4-per-core across 8 NeuronCores; w/b replicated. Per core and batch:
  T   = feats[b]^T                (PE transposes, [F=128, N=1024] in SBUF)
  vwT = w^T-contract:  vwT[g,n] = sum_f w[f,g] T[f,n]      (matmul, lhsT=w)
  y   = vwT^T-contract: y[n,m] = sum_g vwT[g,n] T[g,m] + b (matmul, lhsT=vwT)
Output DMA (16 MiB/core) is the roofline. Matmuls run as float32r (single-pass
fp32, ~2e-4 rel err) to keep the tensor engine off the critical path. Stores
go out in 2 MiB half-batches alternating between the SP and ACT HWDGE rings;
feats prefetches ride the GpSimd SWDGE path so they never block stores.
"""

import numpy as np

import concourse.mybir as mybir
import concourse.tile as tile
from concourse import bacc
from concourse.tile import add_dep_helper
from concourse.bass_utils import run_bass_kernel_spmd
from concourse.masks import make_identity

B, N, F = 32, 1024, 128
N_CORES = 8
BPC = B // N_CORES  # batches per core
P = 128             # partitions
NT = N // P         # row tiles per batch
H = 512             # matmul moving-dim chunk (PSUM bank limit for 4-byte)

F32 = mybir.dt.float32
MM_DT = mybir.dt.float32r  # matmul compute dtype (1 cyc/row vs 4 for float32)


def build_nc(mm_dt=MM_DT):
    nc = bacc.Bacc("TRN2", target_bir_lowering=False, debug=False,
                   num_devices=N_CORES)
    feats_d = nc.dram_tensor("feats", [BPC, N, F], F32, kind="ExternalInput")
    w_d = nc.dram_tensor("w", [F, F], F32, kind="ExternalInput")
    b_d = nc.dram_tensor("b", [1], F32, kind="ExternalInput")
    y_d = nc.dram_tensor("y", [BPC, N, N], F32, kind="ExternalOutput")

    with tile.TileContext(nc) as tc:
        with (
            tc.tile_pool(name="const", bufs=1) as cpool,
            tc.tile_pool(name="fload", bufs=BPC) as fpool,
            tc.tile_pool(name="tbuf", bufs=2) as tpool,
            tc.tile_pool(name="vwbuf", bufs=2) as vpool,
            tc.tile_pool(name="ybuf", bufs=4) as ypool,
            tc.tile_pool(name="ps_t", bufs=2, space="PSUM") as ps_t,
            tc.tile_pool(name="ps_y", bufs=2, space="PSUM") as ps_y,
        ):
            # Tiny loads first on each ring so mm1's weights aren't stuck
            # behind megabyte feats transfers in the ring FIFO.
            ident = cpool.tile([P, P], F32)
            make_identity(nc, ident[:])
            w_s = cpool.tile([F, F], F32)
            nc.gpsimd.dma_start(w_s[:], w_d[:])
            bias_s = cpool.tile([P, 1], F32)
            nc.gpsimd.dma_start(bias_s[:1, :], b_d[None, :])
            nc.gpsimd.partition_broadcast(bias_s[:], bias_s[:1, :])
            # matmul operands must be written in the matmul dtype (fp32r
            # rounding happens at the producing instruction per the BIR
            # verifier), so keep fp32r copies of w / T / vwT in SBUF.
            w_r = cpool.tile([F, F], mm_dt, tag="w_r")
            nc.vector.tensor_copy(w_r[:], w_s[:])

            # PE sits idle for ~7us waiting on the first feats load; spend
            # that window on dummy matmuls so the HAM clock gate is already
            # at 2.4 GHz when real work arrives (cold PE runs at 1.2 GHz).
            warm_ps = ps_t.tile([P, NT, P], F32, tag="t_ps")
            for wi in range(28):
                nc.tensor.matmul(warm_ps[:, wi % NT, :], ident[:], ident[:])

            # feats, contiguous layout: f_t[p, j, f] = feats[bi, p*NT + j, f]
            # (4 KiB contiguous per partition -> full-rate DMA). Batch 0 is
            # split in halves across both HWDGE rings to cut time-to-first-
            # transpose; later batches prefetch via SWDGE off the store path.
            f_tiles = []
            prev_loads = []
            for bi in range(BPC):
                f_t = fpool.tile([P, NT, F], F32, tag="f_t")
                fsrc = feats_d[bi].rearrange("(p j) f -> p j f", j=NT)
                if bi == 0:
                    prev_loads = []
                    for qi in range(4):
                        eng = nc.sync if qi % 2 == 0 else nc.scalar
                        dq = eng.dma_start(
                            f_t[:, qi * 2:(qi + 1) * 2, :],
                            fsrc[:, qi * 2:(qi + 1) * 2, :],
                        )
                        if qi < 2:
                            prev_loads.append(dq.ins)
                else:
                    # SWDGE path: own queues, so prefetches never head-of-line
                    # block the HWDGE store rings. Chained behind the prior
                    # load: the SDMA engines round-robin between queues at
                    # packet granularity, so concurrent loads would stretch
                    # batch 0's completion.
                    d = nc.gpsimd.dma_start(f_t[:], fsrc)
                    for pd in prev_loads:
                        add_dep_helper(d.ins, pd, reason="sequence feats prefetch")
                    prev_loads = [d.ins]
                f_tiles.append(f_t)

            def emit_transposes(bi):
                """Transpose feats[bi]^T into PSUM, scatter-copy to SBUF."""
                f_t = f_tiles[bi]
                t_ps = ps_t.tile([P, NT, P], F32, tag="t_ps")
                for j in range(NT):
                    nc.tensor.transpose(t_ps[:, j, :], f_t[:, j, :], ident[:])
                t_s = tpool.tile([P, N], mm_dt, tag="t_s")
                t_view = t_s[:].rearrange("g (p j) -> g j p", j=NT)
                nc.vector.tensor_copy(t_view[:, : NT // 2, :], t_ps[:, : NT // 2, :])
                nc.vector.tensor_copy(t_view[:, NT // 2:, :], t_ps[:, NT // 2:, :])
                return t_s

            t_cur = emit_transposes(0)
            for bi in range(BPC):
                t_s = t_cur
                # Transposes for the next batch go to the PE *before* this
                # batch's matmuls so the PE never idles on the T copies.
                if bi + 1 < BPC:
                    t_cur = emit_transposes(bi + 1)

                # vwT[g, n] = sum_f w[f, g] * T[f, n]
                vw_ps = ps_t.tile([P, N], F32, tag="t_ps")
                for h in range(N // H):
                    nc.tensor.matmul(
                        vw_ps[:, h * H:(h + 1) * H],
                        w_r[:],
                        t_s[:, h * H:(h + 1) * H],
                    )
                vw_s = vpool.tile([P, N], mm_dt)
                nc.scalar.activation(
                    vw_s[:, :H], vw_ps[:, :H], mybir.ActivationFunctionType.Copy
                )
                nc.vector.tensor_copy(vw_s[:, H:], vw_ps[:, H:])

                # y rows i*P..(i+1)*P-1 = vw_s[:, i*P:(i+1)*P].T @ T (+ bias)
                half_tiles = NT // 2
                for half in range(2):
                    y_s = ypool.tile([P, half_tiles, N], F32, tag="y_s")
                    for k in range(half_tiles):
                        i = half * half_tiles + k
                        y_ps = ps_y.tile([P, N], F32)
                        for h in range(N // H):
                            nc.tensor.matmul(
                                y_ps[:, h * H:(h + 1) * H],
                                vw_s[:, i * P:(i + 1) * P],
                                t_s[:, h * H:(h + 1) * H],
                            )
                        if i % 2 == 0:
                            nc.scalar.activation(
                                y_s[:, k, :], y_ps[:],
                                mybir.ActivationFunctionType.Identity,
                                bias=bias_s[:],
                            )
                        else:
                            nc.vector.tensor_scalar_add(
                                y_s[:, k, :], y_ps[:], bias_s[:]
                            )

                    dram_half = y_d[bi].rearrange("(i p) m -> p i m", p=P)[
                        :, half * half_tiles:(half + 1) * half_tiles, :
                    ]
                    if bi == 0 and half == 0:
                        # Per-tile chunks: the store stream starts immediately.
                        for qi in range(half_tiles):
                            eng = nc.sync if qi % 2 == 0 else nc.scalar
                            eng.dma_start(
                                dram_half[:, qi:qi + 1, :],
                                y_s[:, qi:qi + 1, :],
                            )
                    elif bi == BPC - 1:
                        # Last batch: quarter-chunks so the tail drains fast.
                        q = half_tiles // 2
                        for qi in range(2):
                            eng = nc.sync if (half + qi) % 2 == 0 else nc.scalar
                            eng.dma_start(
                                dram_half[:, qi * q:(qi + 1) * q, :],
                                y_s[:, qi * q:(qi + 1) * q, :],
                            )
                    else:
                        eng = nc.sync if half == 0 else nc.scalar
                        eng.dma_start(dram_half, y_s[:])

    nc.compile()
    return nc


_NC_CACHE = {}


def _get_nc(mm_dt=MM_DT):
    key = str(mm_dt)
    if key not in _NC_CACHE:
        _NC_CACHE[key] = build_nc(mm_dt)
    return _NC_CACHE[key]


def run_on_cores(feats, w, b, mm_dt=MM_DT, trace=False):
    nc = _get_nc(mm_dt)
    feats = np.ascontiguousarray(feats, dtype=np.float32)
    w = np.ascontiguousarray(w, dtype=np.float32)
    b = np.ascontiguousarray(b, dtype=np.float32)
    in_maps = [
        {"feats": feats[c * BPC:(c + 1) * BPC], "w": w, "b": b}
        for c in range(N_CORES)
    ]
    res = run_bass_kernel_spmd(nc, in_maps, core_ids=list(range(N_CORES)),
                               trace=trace)
    y = np.concatenate([res.results[c]["y"] for c in range(N_CORES)], axis=0)
    return y, res


def kernel(adjMs, feats, w, b):
    y, _ = run_on_cores(feats, w, b)
    feats = np.asarray(feats, dtype=np.float32)
    return y, feats
